# revision 9
# baseline (speedup 1.0000x reference)
"""Data-parallel Trainium kernel for nn_BasicModel_63848983823018.

Pure data parallel over batch (per sharding hint): batch 4096 is split into
8 shards of 512 across the 8 NeuronCores; weights are replicated. Each core
runs the full per-sample quantum-circuit forward (state evolution, branch
expansion, measurement) on its shard; host gathers and reassembles.

The device graph is entirely REAL-valued (the neuron backend has no
complex dtype support): the state is carried as separate re/im planes, Rz
phase diagonals and the measurement Gram matrix are expanded into real
arithmetic, and the CNOT permutations are applied as constant 32x32
permutation matmuls. Complex64 output is assembled on host.

Self-contained: all circuit constants are derived here from first
principles — nothing is read from disk.
"""

import numpy as np
import jax
import jax.numpy as jnp

NQ = 5
DIM = 1 << NQ  # 32


def _bar(*ts):
    ts = jax.lax.optimization_barrier(tuple(ts))
    return ts if len(ts) > 1 else ts[0]

NB = 8
N_CORES = 8
NBR = 16  # branches after entangle
ENT_PAIRS = [((0, 0), (1, 0)), ((1, 4), (0, 4)), ((0, 1), (1, 3)), ((1, 2), (0, 2))]


def _zdiag(q):
    return 1 - 2 * ((np.arange(DIM) >> (NQ - 1 - q)) & 1)


def _cnot_tau(L):
    k = np.arange(DIM)
    tau = k.copy()
    for i in range(NQ):
        c, t = i, (i + L) % NQ
        cshift = NQ - 1 - c
        tm = 1 << (NQ - 1 - t)
        sigma = k ^ (((k >> cshift) & 1) * tm)
        tau = tau[sigma]
    return tau


def _perm_matrix(tau):
    # out[i] = in[tau[i]]  =>  out = in @ P.T with P[i, tau[i]] = 1
    P = np.zeros((DIM, DIM), dtype=np.float32)
    P[np.arange(DIM), tau] = 1.0
    return P


PM1 = _perm_matrix(_cnot_tau(1))
PM2 = _perm_matrix(_cnot_tau(2))

OBS_R = np.array([(-1) ** bin(s).count("1") for s in range(DIM)], dtype=np.float32)

_A0 = np.array([1.0, -1j]) / (2**0.5 * (-1j) ** 0.5)
_ALPHA = _A0.copy()
for _ in range(3):
    _ALPHA = np.kron(_ALPHA, _A0)
_C = np.conj(_ALPHA)[:, None] * _ALPHA[None, :]  # (16,16) complex
CR = np.ascontiguousarray(_C.real.astype(np.float32))
CI = np.ascontiguousarray(_C.imag.astype(np.float32))


def _ent_factors():
    facs = []
    for (b1, q1), (b2, q2) in ENT_PAIRS:
        f = np.ones((NB, DIM), dtype=np.float32)
        f[b1] = f[b1] * _zdiag(q1)
        f[b2] = f[b2] * _zdiag(q2)
        facs.append(f)
    return np.stack(facs)  # (4, NB, DIM) real +-1


FAC = _ent_factors()

# Rz sign matrix: ang[s] = sum_q 0.5 * t_q * (2*bit_q(s) - 1)
SGN = np.zeros((DIM, NQ), dtype=np.float32)
for s in range(DIM):
    for q in range(NQ):
        SGN[s, q] = 2.0 * ((s >> (NQ - 1 - q)) & 1) - 1.0


def _krons_flat(mats):
    # mats (BN, NQ, 2, 2) -> (BN, DIM, DIM), qubit 0 outermost.
    # Built with single-batch-dim reshapes + elementwise broadcasting only.
    BN = mats.shape[0]
    out = mats[:, 0]  # (BN, 2, 2)
    for q in range(1, NQ):
        a = out.shape[1]
        nxt = mats[:, q]  # (BN,2,2)
        out = (
            out[:, :, None, :, None] * nxt[:, None, :, None, :]
        ).reshape(BN, 2 * a, 2 * a)
    return out


def _ry_mat_flat(theta):
    # theta (BN, NQ) -> real U (BN, DIM, DIM)
    h = theta * 0.5
    c, s = jnp.cos(h), jnp.sin(h)
    mats = jnp.stack([jnp.stack([c, -s], -1), jnp.stack([s, c], -1)], -2)
    return _krons_flat(mats)


def _ry_matT_flat(theta):
    # builds U^T directly: kron of per-qubit transposes, no transpose op
    h = theta * 0.5
    c, s = jnp.cos(h), jnp.sin(h)
    mats = jnp.stack([jnp.stack([c, s], -1), jnp.stack([-s, c], -1)], -2)
    return _krons_flat(mats)


def _apply_ry(re, im, U):
    # state (BN,K,32), U (BN,32,32): state' = state @ U^T
    Ut = jnp.transpose(U, (0, 2, 1))
    return jnp.matmul(re, Ut), jnp.matmul(im, Ut)


def _apply_perm(re, im, pm):
    # fixed 32x32 permutation matrix: state' = state @ pm^T
    return jnp.matmul(re, pm.T), jnp.matmul(im, pm.T)


def _apply_rz(re, im, theta, sgn):
    # theta (BN, NQ): ang = 0.5 * theta @ SGN^T  (BN, DIM)
    ang = jnp.matmul(theta, sgn.T) * 0.5
    dr = jnp.cos(ang)[:, None, :]
    di = jnp.sin(ang)[:, None, :]
    re2 = re * dr - im * di
    im2 = re * di + im * dr
    return re2, im2


def _forward_real(x, w0, w1a, w1b, w1c, w2, w3a, w3b, w3c):
    B = x.shape[0]
    BN = B * NB
    X = x.reshape(BN, NQ)
    # weights (1,NB,1,NQ,1) -> broadcast to (BN, NQ)
    wflat = [
        jnp.broadcast_to(w.reshape(1, NB, NQ), (B, NB, NQ)).reshape(BN, NQ)
        for w in (w0, w1a, w1b, w1c, w2, w3a, w3b, w3c)
    ]
    w0s, w1as, w1bs, w1cs, w2s, w3as, w3bs, w3cs = wflat
    sgn = jnp.asarray(SGN)
    pm1 = jnp.asarray(PM1)
    pm2 = jnp.asarray(PM2)
    obs = jnp.asarray(OBS_R)
    fac = jnp.asarray(FAC)  # (4, NB, DIM)
    cr = jnp.asarray(CR)
    ci = jnp.asarray(CI)

    Ux = _ry_mat_flat(X)  # (BN,32,32), used 4x
    Uxt = jnp.transpose(Ux, (0, 2, 1))
    Uxt = _bar(Uxt)

    # state (BN, K, 32)
    re = jnp.zeros((BN, 1, DIM), jnp.float32).at[..., 0].set(1.0)
    im = jnp.zeros((BN, 1, DIM), jnp.float32)

    re, im = jnp.matmul(re, Uxt), jnp.matmul(im, Uxt)          # Ry_data
    re, im = _apply_ry(re, im, _ry_mat_flat(w0s))              # fR0
    re, im = jnp.matmul(re, Uxt), jnp.matmul(im, Uxt)          # Ry_data
    re, im = _apply_rz(re, im, w1as, sgn)                      # fR1
    re, im = _apply_ry(re, im, _ry_mat_flat(w1bs))
    re, im = _apply_rz(re, im, w1cs, sgn)
    re, im = _apply_perm(re, im, pm1)                          # CNOT L=1
    re, im = jnp.matmul(re, Uxt), jnp.matmul(im, Uxt)          # Ry_data
    re, im = _bar(re, im)
    # Entangle: 1 -> 16 branches; factor for branch j is the product of the
    # chosen gate masks, per (n, s): mask (16, NB, DIM) host-precomputable.
    mask = np.ones((NBR, NB, DIM), dtype=np.float32)
    for j in range(NBR):
        for g in range(4):
            if (j >> g) & 1:
                mask[j] = mask[j] * FAC[g]
    maskj = jnp.asarray(
        np.broadcast_to(mask.transpose(1, 0, 2), (NB, NBR, DIM)).copy()
    )  # (NB, 16, DIM)
    mflat = jnp.broadcast_to(maskj[None], (B, NB, NBR, DIM)).reshape(BN, NBR, DIM)
    re = re * mflat  # (BN,1,32) * (BN,16,32) -> (BN,16,32)
    im = im * mflat
    re, im = _bar(re, im)
    re, im = _apply_ry(re, im, _ry_mat_flat(w2s))              # fR2
    re, im = _bar(re, im)
    re, im = jnp.matmul(re, Uxt), jnp.matmul(im, Uxt)          # Ry_data
    re, im = _bar(re, im)
    re, im = _apply_rz(re, im, w3as, sgn)                      # fR3
    re, im = _apply_ry(re, im, _ry_mat_flat(w3bs))
    re, im = _apply_rz(re, im, w3cs, sgn)
    re, im = _apply_perm(re, im, pm2)                          # CNOT L=2
    re, im = _bar(re, im)

    # measurement M[j,k] = sum_s conj(S_j) obs S_k (obs real +-1)
    er = re * obs
    ei = im * obs
    ret = jnp.transpose(re, (0, 2, 1))
    imt = jnp.transpose(im, (0, 2, 1))
    Mr = jnp.matmul(er, ret) + jnp.matmul(ei, imt)  # (BN,16,16)
    Mi = jnp.matmul(er, imt) - jnp.matmul(ei, ret)
    Mr, Mi = _bar(Mr, Mi)
    Mr = Mr.reshape(B, NB, NBR, NBR)
    Mi = Mi.reshape(B, NB, NBR, NBR)
    Pr, Pi = Mr[:, 0], Mi[:, 0]
    for n in range(1, NB):
        nr = Pr * Mr[:, n] - Pi * Mi[:, n]
        ni = Pr * Mi[:, n] + Pi * Mr[:, n]
        Pr, Pi = nr, ni
    # O = sum_jk (CR*Pr - CI*Pi)
    Ow = (Pr * cr[None] - Pi * ci[None]).reshape(B, NBR * NBR)
    O = jnp.sum(Ow, axis=1)
    return re, im, O.astype(jnp.float32)


def _ry_matT_np(theta):
    # numpy: (N, NQ) -> U^T (N, DIM, DIM) float32
    h = theta.astype(np.float64) * 0.5
    c, s = np.cos(h), np.sin(h)
    # per-qubit transposed 2x2: [[c, s], [-s, c]]
    out = np.stack([np.stack([c[:, 0], s[:, 0]], -1),
                    np.stack([-s[:, 0], c[:, 0]], -1)], -2)
    for q in range(1, NQ):
        m = np.stack([np.stack([c[:, q], s[:, q]], -1),
                      np.stack([-s[:, q], c[:, q]], -1)], -2)
        a = out.shape[1]
        out = (out[:, :, None, :, None] * m[:, None, :, None, :]).reshape(
            -1, 2 * a, 2 * a
        )
    return out.astype(np.float32)


_STAGES = None


def _mask_flat_np(B):
    mask = np.ones((NBR, NB, DIM), dtype=np.float32)
    for j in range(NBR):
        for g in range(4):
            if (j >> g) & 1:
                mask[j] = mask[j] * FAC[g]
    m = np.broadcast_to(mask.transpose(1, 0, 2), (NB, NBR, DIM))
    return np.broadcast_to(m[None], (B, NB, NBR, DIM)).reshape(B * NB, NBR, DIM).copy()


def _get_stages():
    global _STAGES
    if _STAGES is None:
        sgn = jnp.asarray(SGN)
        pm1t = jnp.asarray(PM1.T.copy())
        pm2t = jnp.asarray(PM2.T.copy())
        obs = jnp.asarray(OBS_R)
        cr = jnp.asarray(CR)
        ci = jnp.asarray(CI)

        def s_ux(x):
            return _ry_matT_flat(x.reshape(-1, NQ))

        def s_wmat(w):
            return _ry_matT_flat(w)

        def s_pre(Uxt, U0t, U1bt, w1as, w1cs):
            BN = Uxt.shape[0]
            re = jnp.zeros((BN, 1, DIM), jnp.float32).at[..., 0].set(1.0)
            im = jnp.zeros((BN, 1, DIM), jnp.float32)
            re, im = jnp.matmul(re, Uxt), jnp.matmul(im, Uxt)
            re, im = jnp.matmul(re, U0t), jnp.matmul(im, U0t)
            re, im = jnp.matmul(re, Uxt), jnp.matmul(im, Uxt)
            re, im = _apply_rz(re, im, w1as, sgn)
            re, im = jnp.matmul(re, U1bt), jnp.matmul(im, U1bt)
            re, im = _apply_rz(re, im, w1cs, sgn)
            re, im = jnp.matmul(re, pm1t), jnp.matmul(im, pm1t)
            re, im = jnp.matmul(re, Uxt), jnp.matmul(im, Uxt)
            return re, im

        def s_mask(re, im, mflat):
            return re * mflat, im * mflat

        def s_post1(re, im, U2t, Uxt):
            re, im = jnp.matmul(re, U2t), jnp.matmul(im, U2t)
            re, im = jnp.matmul(re, Uxt), jnp.matmul(im, Uxt)
            return re, im

        def s_post2(re, im, U3bt, w3as, w3cs):
            re, im = _apply_rz(re, im, w3as, sgn)
            re, im = jnp.matmul(re, U3bt), jnp.matmul(im, U3bt)
            re, im = _apply_rz(re, im, w3cs, sgn)
            re, im = jnp.matmul(re, pm2t), jnp.matmul(im, pm2t)
            return re, im

        def s_meas(re, im):
            er = re * obs
            ei = im * obs
            ret = jnp.transpose(re, (0, 2, 1))
            imt = jnp.transpose(im, (0, 2, 1))
            Mr = jnp.matmul(er, ret) + jnp.matmul(ei, imt)
            Mi = jnp.matmul(er, imt) - jnp.matmul(ei, ret)
            return Mr, Mi

        def s_obs(Mr, Mi):
            Bs = Mr.shape[0] // NB
            Mr = Mr.reshape(Bs, NB, NBR, NBR)
            Mi = Mi.reshape(Bs, NB, NBR, NBR)
            Pr, Pi = Mr[:, 0], Mi[:, 0]
            for n in range(1, NB):
                nr = Pr * Mr[:, n] - Pi * Mi[:, n]
                ni = Pr * Mi[:, n] + Pi * Mr[:, n]
                Pr, Pi = nr, ni
            Ow = (Pr * cr[None] - Pi * ci[None]).reshape(Bs, NBR * NBR)
            return jnp.sum(Ow, axis=1).astype(jnp.float32)

        _STAGES = {k: jax.pmap(v) for k, v in [
            ("ux", s_ux), ("wmat", s_wmat), ("pre", s_pre), ("mask", s_mask),
            ("post1", s_post1), ("post2", s_post2), ("meas", s_meas), ("obs", s_obs),
        ]}
    return _STAGES


def kernel(**inputs):
    x = np.asarray(inputs["x"], dtype=np.float32)
    wnames = ["w0", "w1a", "w1b", "w1c", "w2", "w3a", "w3b", "w3c"]
    ws = {n: np.asarray(inputs[n], dtype=np.float32).reshape(NB, NQ) for n in wnames}

    n = min(N_CORES, len(jax.devices()))
    B = x.shape[0]
    Bs = B // n
    BNs = Bs * NB
    xs = x.reshape(n, BNs, NQ)

    st = _get_stages()
    # per-n weight matrices, broadcast over the shard batch
    def wrep(w):  # (NB,NQ) -> (n, BNs, NQ)
        t = np.broadcast_to(w[None], (Bs, NB, NQ)).reshape(BNs, NQ)
        return np.broadcast_to(t[None], (n, BNs, NQ)).copy()

    Uxt = _ry_matT_np(x.reshape(B * NB, NQ)).reshape(n, BNs, DIM, DIM)

    def wmatrep(w):
        t = _ry_matT_np(w)  # (NB,32,32)
        t = np.broadcast_to(t[None], (Bs, NB, DIM, DIM)).reshape(BNs, DIM, DIM)
        return np.broadcast_to(t[None], (n, BNs, DIM, DIM)).copy()

    U0t = wmatrep(ws["w0"])
    U1bt = wmatrep(ws["w1b"])
    U2t = wmatrep(ws["w2"])
    U3bt = wmatrep(ws["w3b"])
    w1as = wrep(ws["w1a"]); w1cs = wrep(ws["w1c"])
    w3as = wrep(ws["w3a"]); w3cs = wrep(ws["w3c"])

    re, im = st["pre"](Uxt, U0t, U1bt, w1as, w1cs)
    mflat = _mask_flat_np(Bs)
    mrep = np.broadcast_to(mflat[None], (n, BNs, NBR, DIM)).copy()
    re, im = st["mask"](re, im, mrep)
    re, im = st["post1"](re, im, U2t, Uxt)
    re, im = st["post2"](re, im, U3bt, w3as, w3cs)
    Mr, Mi = st["meas"](re, im)
    O_sh = st["obs"](Mr, Mi)

    re_np = np.asarray(re).reshape(B, NB, NBR, DIM)
    im_np = np.asarray(im).reshape(B, NB, NBR, DIM)
    O = np.asarray(O_sh).reshape(B)
    state = (re_np + 1j * im_np).astype(np.complex64)[..., None]
    return state, O.astype(np.float32)
